# revision 6
# baseline (speedup 1.0000x reference)
"""MDCT kernel for Trainium2 (8 NeuronCores, batch-parallel), folded DCT-IV form.

Math: frame f (hop N=1024, len 2N, center-padded) folds to an N-vector u and
out[f] = DCT-IV(u).  With x2 = x.reshape(1024, 1024) and y1 = w[:N]*x2[r],
y2 = w[N:]*x2[r] (per-row windowing):
    u[f, m]      = -y2[f, 511-m] - y2[f, 512+m]      (m < 512,  row f)
    u[f, 512+p]  =  y1[f-1, p]   - y1[f-1, 1023-p]   (p < 512,  row f-1)
so each x2 row r yields uLo[r] (frame r) and uHi[r] (frame r+1), and
    out[f, k] = sum_m u[f, m] * D4[m, k],   D4 = sqrt(2/N) DCT-IV matrix.

This halves the matmul contraction (1024 vs 2048) vs the direct form.  The
fold runs on the vector engine (reversals are negative-stride APs), u is
transposed on the PE in bf16 (1 cyc/row), and the DCT matmuls run in bf16
(same PE rate as fp32r, but half the weight-matrix HBM traffic).

Schedule notes (v2, derived from the 53.8us baseline NTFF profile):
- every dma_start costs ~600 ns of issue time on its engine's queue; the
  baseline's 43 DMAs burned 26 us of Sync and gated DCT start at 15.5 us.
  v2 batches fills (x0+x7 fused via a strided 3D AP, D in 4 two-chunk
  DMAs) and issues output stores on the idle GpSimd queue so they never
  queue behind fill transfers on the Sync hardware queue.
- fill order: wb (4 KB), x0+x7, D45, D67, x1, D01, D23, x2..x6 - the DCT
  gate (full D) lands ~11 us, and x1 lands in time for fold(1) to feed
  dct_tile(1) right after tile 0.
- window rows are DMA'd once ([2, 1024]) and replicated to 128 partitions
  by GpSimd partition_broadcast - 4 KB instead of 512 KB on the critical
  fill path.
- fold(7) runs first (x7 arrives in the first x DMA) so last_frame's 8
  matmuls (frame 1024, uHi-only) fill the PE idle slot while D0-3 is
  still in flight, instead of extending the busy DCT phase.
- warmup transposes are front-loaded so the HAM clock gate ramps to
  2.4 GHz by ~4 us (ramp takes ~3.5 us of sustained PE activity; a
  >3.4 us PE idle re-throttles to 1.2 GHz).
- PE program order fold(r) -> dct_tile(r) software-pipelines fold
  transposes into the chained-matmul stream; PSUM->SBUF copies split
  ACT (lo half) / DVE (hi half) per tile; tile 7's store is split in
  halves so the tail copy+DMA pipelines.
"""

import numpy as np
import ml_dtypes

import concourse.bass as bass
import concourse.bacc as bacc
import concourse.mybir as mybir
import concourse.tile as tile
from concourse import masks
from concourse.bass_utils import run_bass_kernel_spmd

B = 8
T = 1 << 20
R = 1024          # rows of x2 per channel (T // hop)
CN = 1024         # row width (hop) = N
NF = 1025         # output frames
NK = 1024         # output bins
F32 = mybir.dt.float32
BF16 = mybir.dt.bfloat16

_NC_CACHE = None
_CONST_CACHE = None


def build_nc() -> bass.Bass:
    nc = bacc.Bacc("TRN2", target_bir_lowering=False, debug=False)
    x = nc.dram_tensor("x", [R, CN], BF16, kind="ExternalInput").ap()
    wb = nc.dram_tensor("wb", [1, 2 * CN], BF16, kind="ExternalInput").ap()
    d4 = nc.dram_tensor("d4", [8, 128, NK], BF16, kind="ExternalInput").ap()
    out = nc.dram_tensor("out", [NF, NK], BF16, kind="ExternalOutput").ap()

    # x viewed as [p, rowblock, col] so one DMA can fetch non-adjacent
    # 128-row blocks (strided on the middle dim).
    xv = x.rearrange("(a p) c -> p a c", p=128)
    dv = d4.rearrange("a p c -> p a c")

    with tile.TileContext(nc) as tc:
        with (
            tc.tile_pool(name="persist", bufs=1) as persist,
            tc.tile_pool(name="xin", bufs=1) as xin,
            tc.tile_pool(name="ypool", bufs=6) as ypool,
            tc.tile_pool(name="upool", bufs=4) as upool,
            tc.tile_pool(name="outp", bufs=4) as outp,
            tc.tile_pool(name="tps", bufs=4, space="PSUM") as tps,
            tc.tile_pool(name="mmps", bufs=4, space="PSUM") as mmps,
        ):
            wtile = persist.tile([1, 2 * CN], BF16)
            wcat = persist.tile([128, 2 * CN], BF16)

            ident = persist.tile([128, 128], BF16)
            masks.make_identity(nc, ident[:])

            dt = persist.tile([128, 8, NK], BF16)
            ulot = persist.tile([128, 4, R], BF16)
            uhit = persist.tile([128, 4, NF], BF16)
            nc.vector.memset(uhit[:, :, 0:1], 0.0)

            # x07: row blocks 0 and 7 (one strided DMA); x1..x6 single blocks.
            x07 = xin.tile([128, 2, CN], BF16, name="x07")
            xts = [xin.tile([128, CN], BF16, name=f"xt{i}") for i in range(1, 7)]

            def xsl(r):
                if r == 0:
                    return x07[:, 0, :]
                if r == 7:
                    return x07[:, 1, :]
                return xts[r - 1][:]

            # PE warmup: keep the HAM clock gate fed from t~0 so it ramps
            # to 2.4 GHz before the folds' transposes and the DCT stream.
            warm = tps.tile([128, 512], BF16, tag="tp")
            for _ in range(48):
                nc.tensor.transpose(warm[:, 0:128], ident[:], ident[:])

            # Fill DMAs (Sync queue), critical-path first.
            nc.sync.dma_start(wtile[:], wb)
            nc.sync.dma_start(x07[:], xv[:, 0:8:7, :])
            nc.gpsimd.partition_broadcast(wcat[:], wtile[:])
            w1 = wcat[:, 0:CN]
            w2n = wcat[:, CN:2 * CN]
            nc.sync.dma_start(dt[:, 4:6, :], dv[:, 4:6, :])
            nc.sync.dma_start(dt[:, 6:8, :], dv[:, 6:8, :])
            nc.sync.dma_start(xts[0][:], xv[:, 1, :])
            nc.sync.dma_start(dt[:, 0:2, :], dv[:, 0:2, :])
            nc.sync.dma_start(dt[:, 2:4, :], dv[:, 2:4, :])
            for r in range(2, 7):
                nc.sync.dma_start(xts[r - 1][:], xv[:, r, :])

            def fold(r: int):
                xt = xsl(r)
                un = upool.tile([128, CN], BF16)
                r0 = r * 128
                # hi half first: fold(7)'s uHi feeds last_frame early.
                y1 = ypool.tile([128, CN], BF16, tag="y1")
                nc.vector.tensor_tensor(y1[:], xt, w1, mybir.AluOpType.mult)
                # uHi[p] = y1[p] - y1[1023-p]
                nc.vector.tensor_tensor(
                    un[:, 512:1024], y1[:, 0:512], y1[:, 1023:511:-1],
                    mybir.AluOpType.subtract,
                )
                phi = tps.tile([128, 512], BF16, tag="tp")
                for ci in range(4):
                    nc.tensor.transpose(
                        phi[:, ci * 128:(ci + 1) * 128],
                        un[:, 512 + ci * 128:512 + (ci + 1) * 128], ident[:],
                    )
                nc.scalar.copy(uhit[:, 0:4, 1 + r0:1 + r0 + 128], phi[:])
                y2n = ypool.tile([128, CN], BF16, tag="y2n")
                nc.vector.tensor_tensor(y2n[:], xt, w2n, mybir.AluOpType.mult)
                # uLo[m] = y2n[511-m] + y2n[512+m]   (y2n = -w2*x)
                nc.vector.tensor_tensor(
                    un[:, 0:512], y2n[:, 511::-1], y2n[:, 512:1024],
                    mybir.AluOpType.add,
                )
                plo = tps.tile([128, 512], BF16, tag="tp")
                for ci in range(4):
                    nc.tensor.transpose(
                        plo[:, ci * 128:(ci + 1) * 128],
                        un[:, ci * 128:(ci + 1) * 128], ident[:],
                    )
                nc.vector.tensor_copy(ulot[:, 0:4, r0:r0 + 128], plo[:])

            def wslice(ci, f0):
                if ci < 4:
                    return ulot[:, ci, f0:f0 + 128]
                return uhit[:, ci - 4, f0:f0 + 128]

            CHAIN = (4, 5, 6, 7, 0, 1, 2, 3)

            def last_frame():
                # f=1024: only the uHi half (row 1023) contributes.
                pa = mmps.tile([1, 512], F32, tag="mm")
                pb = mmps.tile([1, 512], F32, tag="mm")
                for ci in range(4):
                    wsl = uhit[:, ci, 1024:1025]
                    nc.tensor.matmul(
                        pa[:], wsl, dt[:, 4 + ci, 0:512],
                        start=(ci == 0), stop=(ci == 3),
                    )
                    nc.tensor.matmul(
                        pb[:], wsl, dt[:, 4 + ci, 512:1024],
                        start=(ci == 0), stop=(ci == 3),
                    )
                ot = outp.tile([1, NK], BF16, tag="ot_last")
                nc.scalar.copy(ot[:, 0:512], pa[:])
                nc.vector.tensor_copy(ot[:, 512:1024], pb[:])
                nc.gpsimd.dma_start(out[1024:1025, :], ot[:])

            def dct_tile(j: int):
                f0 = j * 128
                ot = outp.tile([128, NK], BF16)
                pa = mmps.tile([128, 512], F32, tag="mm")
                for ci in CHAIN:
                    nc.tensor.matmul(
                        pa[:], wslice(ci, f0), dt[:, ci, 0:512],
                        start=(ci == CHAIN[0]), stop=(ci == CHAIN[-1]),
                    )
                nc.scalar.copy(ot[:, 0:512], pa[:])
                if j == 7:
                    nc.gpsimd.dma_start(out[f0:f0 + 128, 0:512], ot[:, 0:512])
                pb = mmps.tile([128, 512], F32, tag="mm")
                for ci in CHAIN:
                    nc.tensor.matmul(
                        pb[:], wslice(ci, f0), dt[:, ci, 512:1024],
                        start=(ci == CHAIN[0]), stop=(ci == CHAIN[-1]),
                    )
                nc.vector.tensor_copy(ot[:, 512:1024], pb[:])
                if j == 7:
                    nc.gpsimd.dma_start(out[f0:f0 + 128, 512:1024], ot[:, 512:1024])
                else:
                    nc.gpsimd.dma_start(out[f0:f0 + 128, :], ot[:])

            fold(0)
            fold(7)
            last_frame()
            dct_tile(0)
            for r in range(1, 7):
                fold(r)
                dct_tile(r)
            dct_tile(7)

    return nc


def make_consts(window: np.ndarray):
    w = window.astype(np.float64)
    wb = np.concatenate([w[:CN], -w[CN:]]).reshape(1, 2 * CN).astype(ml_dtypes.bfloat16)
    m = np.arange(NK, dtype=np.float64)[:, None]
    k = np.arange(NK, dtype=np.float64)[None, :]
    d = (np.sqrt(2.0 / NK) * np.cos(np.pi / NK * (m + 0.5) * (k + 0.5)))
    d4 = d.astype(ml_dtypes.bfloat16).reshape(8, 128, NK)
    return wb, d4


def _get_nc() -> bass.Bass:
    global _NC_CACHE
    if _NC_CACHE is None:
        _NC_CACHE = build_nc()
        _NC_CACHE.compile()
    return _NC_CACHE


def run_spmd(x: np.ndarray, window: np.ndarray, **kwargs):
    """Shard, run on 8 cores, return (stacked output, BassKernelResults)."""
    global _CONST_CACHE
    if _CONST_CACHE is None or _CONST_CACHE[0] != window.tobytes():
        _CONST_CACHE = (window.tobytes(), make_consts(window))
    wb, d4 = _CONST_CACHE[1]
    in_maps = [
        {"x": np.ascontiguousarray(
            x[b].reshape(R, CN).astype(ml_dtypes.bfloat16)),
         "wb": wb, "d4": d4}
        for b in range(B)
    ]
    res = run_bass_kernel_spmd(nc=_get_nc(), in_maps=in_maps,
                               core_ids=list(range(B)), **kwargs)
    out = np.stack([res.results[b]["out"].astype(np.float32) for b in range(B)],
                   axis=0)
    return out, res


def kernel(x: np.ndarray, window: np.ndarray) -> np.ndarray:
    out, _ = run_spmd(np.asarray(x), np.asarray(window))
    return out
